# revision 26
# baseline (speedup 1.0000x reference)
"""ConvNearestNeightbor Trainium2 kernel.

out[b, n*C+c, i, j] = max_k |x[b,c,i-r_k,j-c_k] - neighbors[n,c,k]|
over the 9 zero-padded 3x3 shifts (r_k, c_k).

Sharding: 8 cores = 4 batch-groups x 2 num-groups.
Per core: B_LOC=4 batches, N_LOC=16 codebook entries.
Partition layout: (nn in 0..3, c in 0..31) -> 128 partitions, with the
codebook tile index nt in 0..3 selecting n = nt*4+nn.

Design.  The workload is pure elementwise (9 abs-diff produces + 8 max
folds per output element); on TRN2 only ACT and DVE can run it (probed:
no abs_max AluOp in the trn2 codegen, no tensor_tensor/bitwise ops on
GPSIMD, Q7 tensor_scalar measures ~17cyc/elem AND its SBUF streaming
contends the shared DVE port degrading folds ~8x, custom DVE uops are
1x-only, DMA-engine accum supports add not max).  So:
- Host prepares the padded fp16 image (zero borders, replicated over
  the 4 nn partition groups) and the +/- neighbor tables: the device
  spends zero engine time on padding / conversion / negation.
- ACT produces |x-nb| planes in one op each (Abs activation, bias=-nb,
  ~3.7us per 4096-elem plane unit): ~26 units.
- DVE produces ~10 units via tensor_scalar subtract at 4x mode (needs
  even window offsets: k in {0,2,3,5,6,8}) + one batched uint16
  bitwise_and per chain over a wide tile; DVE folds all 32 units with
  tensor_tensor max (fp16 2x mode, ~70us total -- the hard floor).
  Measured equilibrium: ACT ~97us and DVE ~99us, both gapless.
- Ramp/tail: unit 0 runs as two half chains (first and last) so the
  first folds gate on only a quarter of the image load and the tail
  drains at half size.  Loads ride sync + scalar + GPSIMD-SWDGE queues
  in parallel with the tiny neighbor tables first on the HWDGE queues
  (anywhere else their completion sems gate the first compute op);
  mid-kernel stores stay on the idle sync queue (scalar triggers would
  stall the saturated ACT sequencer), the tail store runs as two
  parallel halves on sync + scalar (ACT's queue is drained by then;
  finer splits measure worse).  Output fp16; host upcasts.
"""

import numpy as np

B, C, H, W = 16, 32, 32, 32
NUM = 32
NCORES = 8
BG, NG = 4, 2          # batch groups x num groups
B_LOC = B // BG        # 4
N_LOC = NUM // NG      # 16
NT = N_LOC // 4        # 4 codebook tiles of 4 n each
PH, PW = H + 2, W + 2  # 34 x 34 padded image
HB = B_LOC // 2        # batches per half chain
FD = B_LOC * H * W     # 4096 free elems per full unit
PFD = B_LOC * PH * PW  # 4624 padded free elems

# window start offsets within the padded 34x34 image for the 9 shifts
OFFS = [(1 - r, 1 - c) for r in (-1, 0, 1) for c in (-1, 0, 1)]

# chains: (nt, half) with half=None meaning both halves (FD 4096)
CHAINS = [(0, 0), (1, None), (2, None), (3, None), (0, 1)]
# per-chain DVE-produced planes (subtract + and, 4x -> need even window
# offsets: k in {0,2,3,5,6,8}); the rest go to ACT (Abs+bias, any k).
V_PLANES = [(3, 5, 8), (3, 5), (3, 5), (3, 5, 8), (3, 5, 8)]
A_ORDER = (4, 1, 7, 0, 2, 6, 8)   # ACT emission order (k=8 only if not V)
# fold order: DVE's own planes first, ACT planes in production order
FOLD_ORDER = {
    (3, 5, 8): (3, 5, 8, 4, 1, 7, 0, 2, 6),
    (3, 5): (3, 5, 4, 1, 7, 0, 2, 6, 8),
}

_module_cache = {}


def _build_module():
    import concourse.bacc as bacc
    import concourse.mybir as mybir
    import concourse.tile as tile

    dt = mybir.dt
    Alu = mybir.AluOpType
    AF = mybir.ActivationFunctionType
    MASK16 = 0x7FFF

    nc = bacc.Bacc("TRN2", debug=False)
    xpd = nc.dram_tensor("xpad", [128, PFD], dt.float16, kind="ExternalInput")
    nbp = nc.dram_tensor("nbp", [128, NT * 9], dt.float32, kind="ExternalInput")
    nbn = nc.dram_tensor("nbn", [128, NT * 9], dt.float32, kind="ExternalInput")
    out = nc.dram_tensor(
        "out", [B_LOC, N_LOC * C, H, W], dt.float16, kind="ExternalOutput"
    )

    with tile.TileContext(nc) as tc:
        with (
            tc.tile_pool(name="const", bufs=1) as cpool,
            tc.tile_pool(name="accp", bufs=4) as apool,
            tc.tile_pool(name="dp", bufs=10) as dpool,
            tc.tile_pool(name="vp", bufs=3) as vpool,
        ):
            # padded image: batch quarters spread over three DMA paths so
            # the first half chain (batches 0-1) is gated by only two
            # parallel quarter transfers; batches 2-3 ride the GPSIMD
            # SWDGE (descriptor-gen only on Q7 -- no SBUF port streaming)
            nbpt = cpool.tile([128, NT * 9], dt.float32, tag="nbpt")
            nbnt = cpool.tile([128, NT * 9], dt.float32, tag="nbnt")
            xp = cpool.tile([128, PFD], dt.float16, tag="xp")
            qtr = PFD // 4
            # tiny nb tables first, one per HWDGE queue: they cost the
            # gating image quarters only a ~0.7us trigger delay but their
            # completion sems resolve early and deterministically (via
            # SWDGE they jitter; queued after the quarters they gate the
            # first compute op).
            nc.sync.dma_start(nbpt[:], nbp.ap())
            nc.scalar.dma_start(nbnt[:], nbn.ap())
            nc.sync.dma_start(xp[:, 0:qtr], xpd.ap()[:, 0:qtr])
            nc.scalar.dma_start(xp[:, qtr : 2 * qtr], xpd.ap()[:, qtr : 2 * qtr])
            nc.gpsimd.dma_start(xp[:, 2 * qtr : 3 * qtr], xpd.ap()[:, 2 * qtr : 3 * qtr])
            nc.gpsimd.dma_start(xp[:, 3 * qtr :], xpd.ap()[:, 3 * qtr :])
            xp4 = xp[:].rearrange("p (b h w) -> p b h w", b=B_LOC, h=PH, w=PW)

            out_v = out.ap().rearrange("b (t p) h w -> t p b (h w)", t=NT)

            for nt, h in CHAINS:
                v_ks = V_PLANES[CHAINS.index((nt, h))]
                nb_half = 2 if h is not None else B_LOC  # batches in chain
                fdc = nb_half * H * W
                b0 = 0 if h in (0, None) else HB
                xw = xp4[:, b0 : b0 + nb_half]

                d_tiles = {}

                # ACT planes
                for k in A_ORDER:
                    if k in v_ks:
                        continue
                    a, bcol = OFFS[k]
                    d = dpool.tile([128, fdc], dt.float16, tag="d")
                    d_v = d[:].rearrange(
                        "p (b h w) -> p b h w", b=nb_half, h=H, w=W
                    )
                    col = nt * 9 + k
                    nc.scalar.activation(
                        d_v, xw[:, :, a : a + H, bcol : bcol + W], AF.Abs,
                        bias=nbnt[:, col : col + 1], scale=1.0,
                    )
                    d_tiles[k] = (d, 0)

                # DVE planes: subtracts into one wide tile, single AND
                nv = len(v_ks)
                vd = vpool.tile([128, nv * fdc], dt.float16, tag="vd")
                for i, k in enumerate(v_ks):
                    a, bcol = OFFS[k]
                    seg = vd[:, i * fdc : (i + 1) * fdc]
                    seg_v = seg.rearrange(
                        "p (b h w) -> p b h w", b=nb_half, h=H, w=W
                    )
                    col = nt * 9 + k
                    nc.vector.tensor_scalar(
                        seg_v, xw[:, :, a : a + H, bcol : bcol + W],
                        nbpt[:, col : col + 1], None, Alu.subtract,
                    )
                    d_tiles[k] = (vd, i)
                nc.vector.tensor_scalar(
                    vd[:].bitcast(dt.uint16), vd[:].bitcast(dt.uint16),
                    MASK16, None, Alu.bitwise_and,
                )

                def seg_of(k):
                    t, i = d_tiles[k]
                    if t.shape[1] == fdc:
                        return t[:]
                    return t[:, i * fdc : (i + 1) * fdc]

                order = FOLD_ORDER[v_ks]
                acc = apool.tile([128, fdc], dt.float16, tag="acc")
                nc.vector.tensor_tensor(
                    acc[:], seg_of(order[0]), seg_of(order[1]), Alu.max
                )
                for k in order[2:]:
                    nc.vector.tensor_tensor(
                        acc[:], acc[:], seg_of(k), Alu.max
                    )

                # store fp16.  Mid-kernel stores stay on the idle sync
                # queue (scalar-queue triggers would stall the saturated
                # ACT sequencer); the tail store splits sync + GPSIMD
                # SWDGE so both halves drain in parallel.
                acc_s = acc[:].rearrange("p (b s) -> p b s", b=nb_half)
                dst = out_v[nt][:, b0 : b0 + nb_half]
                hh = nb_half // 2
                if (nt, h) == CHAINS[-1]:
                    # tail store: two parallel halves, one per HWDGE queue
                    # (4-way splitting measures worse -- the extra trigger
                    # latency lands on the critical path)
                    nc.sync.dma_start(dst[:, 0:hh], acc_s[:, 0:hh])
                    nc.scalar.dma_start(dst[:, hh:], acc_s[:, hh:])
                else:
                    nc.sync.dma_start(dst[:, 0:hh], acc_s[:, 0:hh])
                    nc.sync.dma_start(dst[:, hh:], acc_s[:, hh:])

    nc.compile()
    return nc


def _get_module():
    if "nc" not in _module_cache:
        _module_cache["nc"] = _build_module()
    return _module_cache["nc"]


def _prep_core_inputs(x, neighbors, bg, ng):
    """Host-side prep: pad + fp16 + replicate x, rearrange +/- neighbors."""
    xs = x[bg * B_LOC : (bg + 1) * B_LOC]          # (B_LOC, C, H, W) fp32
    xp = np.zeros((C, B_LOC, PH, PW), dtype=np.float16)
    xp[:, :, 1 : 1 + H, 1 : 1 + W] = xs.transpose(1, 0, 2, 3)
    xp = np.tile(xp.reshape(C, PFD), (4, 1))        # (128, PFD)

    nl = neighbors[ng * N_LOC : (ng + 1) * N_LOC]   # (N_LOC, C, 9)
    nbp = (
        nl.reshape(NT, 4, C, 9)
        .transpose(1, 2, 0, 3)
        .reshape(128, NT * 9)
        .astype(np.float32)
    )
    return {
        "xpad": np.ascontiguousarray(xp),
        "nbp": np.ascontiguousarray(nbp),
        "nbn": np.ascontiguousarray(-nbp),
    }


def _run(x, neighbors, trace=False):
    from concourse import bass_utils

    x = np.ascontiguousarray(x, dtype=np.float32)
    neighbors = np.ascontiguousarray(neighbors, dtype=np.float32)
    in_maps = []
    for core in range(NCORES):
        bg, ng = divmod(core, NG)
        in_maps.append(_prep_core_inputs(x, neighbors, bg, ng))
    res = bass_utils.run_bass_kernel_spmd(
        _get_module(), in_maps, core_ids=list(range(NCORES)), trace=trace
    )
    out = np.empty((B, NUM * C, H, W), dtype=np.float32)
    for core in range(NCORES):
        bg, ng = divmod(core, NG)
        out[bg * B_LOC : (bg + 1) * B_LOC, ng * N_LOC * C : (ng + 1) * N_LOC * C] = (
            res.results[core]["out"].astype(np.float32)
        )
    return out, res


def kernel(x, neighbors):
    out, _ = _run(x, neighbors, trace=False)
    return out
